# revision 13
# baseline (speedup 1.0000x reference)
"""XNOR-Net conv2d kernel for Trainium2.

Computes conv2d(sign(x), sign(W), stride=1, pad=1) * alpha for
x:(32,256,56,56) f32, W:(256,256,3,3) f32, alpha:(256,1,1) f32.

Strategy: data-parallel over batch (4 images per core x 8 cores).
Per core, implicit GEMM on the PE array in fp8. sign(x) is +-1 in
fp8e4 (exact); sign(W) is represented as +-0.5 (one-pass DVE compute:
(w>0) - 0.5), with the missing x2 folded into alpha. Products are
+-0.5, accumulated in fp32 PSUM -> half-integers, exact; the final
scale restores integers, so the result is bit-exact vs the reference.

sign(x) lives in SBUF as a zero-padded fp8 image
[128 part = C_in%128, 2 c-groups, 58 rows, 64 row-stride]. Each 3x3
tap is one DoubleRow matmul contracting all 256 input channels
(K = 128 partitions x 2 c-groups): lhsT [128, 2cg, 128co], rhs
[128, 2cg, 8 rows, 56 cols] (shifted window, N=448). 9 taps
accumulate into one PSUM bank; copyback applies 2*alpha.

Pipelining: software-pipelined emission. Image i+1/i+2 chunked loads
and signs are emitted before image i's matmul/store phase so the Sync
DMA queue always has ready loads ahead of copyback-gated stores and
the ACT queue holds only signs. Weight prep is split per output-half:
mt0 transposes run first, img0's mt0 matmuls start while mt1 preps.
"""

import sys

sys.path.insert(0, "/opt/trn_rl_repo")

import numpy as np

import concourse.bass as bass
import concourse.mybir as mybir
from concourse import bacc
from concourse.bass_utils import run_bass_kernel_spmd
from concourse.masks import make_identity
from concourse.tile import TileContext

P = 128
N_CORES = 8
N_IMG = 32
IMG_PER_CORE = N_IMG // N_CORES
C = 256
H = W = 56
HP = 58  # padded rows (0..57)
WS = 64  # row stride of padded buffer (cols 0..57 used, 58+ never read)
CHUNK = 8  # output rows per matmul tile -> N = 8*56 = 448
FP8 = mybir.dt.float8e4

last_result = None  # stash of BassKernelResults for test harnesses


def build_conv_kernel():
    nc = bacc.Bacc()
    x_in = nc.declare_dram_parameter(
        "x", [IMG_PER_CORE, C, H, W], mybir.dt.float32, isOutput=False
    )
    w_in = nc.declare_dram_parameter("w", [C, C, 3, 3], mybir.dt.float32, isOutput=False)
    a_in = nc.declare_dram_parameter("alpha", [C, 1, 1], mybir.dt.float32, isOutput=False)
    y_out = nc.declare_dram_parameter(
        "y", [IMG_PER_CORE, C, H, W], mybir.dt.float32, isOutput=True
    )
    x_ap, w_ap, a_ap, y_ap = x_in[:], w_in[:], a_in[:], y_out[:]

    with TileContext(nc) as tc:
        with (
            tc.tile_pool(name="wpool", bufs=1) as wpool,
            tc.tile_pool(name="xpool", bufs=3) as xpool,
            tc.tile_pool(name="opool", bufs=6) as opool,
            tc.tile_pool(name="pp", bufs=4, space="PSUM") as pp,
        ):
            # warm up the ACT function table while the first DMAs run
            warm = wpool.tile([P, 1], mybir.dt.float32, name="warm")
            nc.vector.memset(warm, 0.0)
            nc.scalar.sign(warm, warm)

            ident = wpool.tile([P, P], mybir.dt.bfloat16, name="ident")
            make_identity(nc, ident)
            alpha_sb = wpool.tile([P, 2], mybir.dt.float32, name="alpha_sb")
            nc.sync.dma_start(
                out=alpha_sb, in_=a_ap.flatten().rearrange("(mt co) -> co mt", co=P)
            )
            # weights carry +-0.5; restore the factor 2 here (on DVE so the
            # ACT queue stays free for x signs)
            nc.vector.tensor_scalar(
                out=alpha_sb,
                in0=alpha_sb,
                scalar1=2.0,
                scalar2=None,
                op0=mybir.AluOpType.mult,
            )

            # [ci_lo, cg, mt, pos, co]
            w_lhsT = wpool.tile([P, 2, 2, 9, P], FP8, name="w_lhsT")

            def emit_wprep(mt):
                for cg in range(2):
                    wsrc = wpool.tile([P, P, 9], mybir.dt.float32, name="wsrc", bufs=2)
                    nc.sync.dma_start(
                        out=wsrc,
                        in_=w_ap[
                            mt * P : (mt + 1) * P, cg * P : (cg + 1) * P
                        ].rearrange("co ci kh kw -> co ci (kh kw)"),
                    )
                    # one-pass half-sign on DVE: (w > 0) - 0.5 -> +-0.5
                    wsgn = wpool.tile([P, P, 9], mybir.dt.bfloat16, name="wsgn", bufs=2)
                    nc.vector.tensor_scalar(
                        out=wsgn,
                        in0=wsrc,
                        scalar1=0.0,
                        scalar2=0.5,
                        op0=mybir.AluOpType.is_gt,
                        op1=mybir.AluOpType.subtract,
                    )
                    for pos in range(9):
                        tp = pp.tile([P, P], mybir.dt.bfloat16, name="tp", bufs=4)
                        nc.tensor.transpose(tp, wsgn[:, :, pos], ident)
                        nc.vector.tensor_copy(out=w_lhsT[:, cg, mt, pos, :], in_=tp)

            xpads = {}

            def emit_loads(img):
                xpad = xpool.tile([P, 2, HP, WS], FP8, name="xpad")
                xpads[img] = xpad
                nc.vector.memset(xpad[:, :, 0, 0:58], 0.0)
                nc.vector.memset(xpad[:, :, HP - 1, 0:58], 0.0)
                nc.vector.memset(xpad[:, :, 1 : HP - 1, 0], 0.0)
                nc.vector.memset(xpad[:, :, 1 : HP - 1, 57], 0.0)
                srcs = []
                for r0 in range(0, H, CHUNK):
                    for cg in range(2):
                        xsrc = xpool.tile(
                            [P, CHUNK, W], mybir.dt.float32, name="xsrc", bufs=14
                        )
                        nc.sync.dma_start(
                            out=xsrc,
                            in_=x_ap[img, cg * P : (cg + 1) * P, r0 : r0 + CHUNK],
                        )
                        srcs.append((r0, cg, xsrc))
                return srcs

            def emit_signs(img, srcs):
                xpad = xpads[img]
                for r0, cg, xsrc in srcs:
                    nc.scalar.sign(
                        xpad[:, cg, r0 + 1 : r0 + 1 + CHUNK, 1 : W + 1], xsrc
                    )

            def emit_mms(img, mts=(0, 1)):
                xpad = xpads[img]
                for h0 in range(0, H, CHUNK):
                    for mt in mts:
                        acc = pp.tile([P, CHUNK * W], mybir.dt.float32, name="acc")
                        k = 0
                        for kh in range(3):
                            for kw in range(3):
                                nc.tensor.matmul(
                                    acc,
                                    w_lhsT[:, :, mt, kh * 3 + kw, :],
                                    xpad[:, :, h0 + kh : h0 + kh + CHUNK, kw : kw + W],
                                    start=(k == 0),
                                    stop=(k == 8),
                                    perf_mode=mybir.MatmulPerfMode.DoubleRow,
                                )
                                k += 1
                        ot = opool.tile([P, CHUNK, W], mybir.dt.float32, name="ot")
                        nc.vector.tensor_scalar_mul(
                            out=ot,
                            in0=acc.rearrange("p (r c) -> p r c", c=W),
                            scalar1=alpha_sb[:, mt : mt + 1],
                        )
                        nc.sync.dma_start(
                            out=y_ap[img, mt * P : (mt + 1) * P, h0 : h0 + CHUNK, :],
                            in_=ot,
                        )

            emit_wprep(0)
            for img in (0, 1):
                srcs = emit_loads(img)
                emit_signs(img, srcs)
            # img0 matmuls split per output-half: mt0 runs on PE while mt1's
            # weights finish prepping
            emit_mms(0, mts=(0,))
            emit_wprep(1)
            emit_mms(0, mts=(1,))
            for img in range(1, IMG_PER_CORE):
                if img + 1 < IMG_PER_CORE:
                    srcs = emit_loads(img + 1)
                    emit_signs(img + 1, srcs)
                emit_mms(img)
    nc.compile()
    return nc


def kernel(x, weight, alpha, trace=False):
    global last_result
    x = np.ascontiguousarray(np.asarray(x, dtype=np.float32))
    weight = np.ascontiguousarray(np.asarray(weight, dtype=np.float32))
    alpha = np.ascontiguousarray(np.asarray(alpha, dtype=np.float32))

    nc = build_conv_kernel()
    in_maps = [
        {
            "x": np.ascontiguousarray(x[i * IMG_PER_CORE : (i + 1) * IMG_PER_CORE]),
            "w": weight,
            "alpha": alpha,
        }
        for i in range(N_CORES)
    ]
    res = run_bass_kernel_spmd(nc, in_maps, list(range(N_CORES)), trace=trace)
    last_result = res
    out = np.concatenate([res.results[i]["y"] for i in range(N_CORES)], axis=0)
    return out.astype(np.float32, copy=False)
